# revision 9
# baseline (speedup 1.0000x reference)
"""Trainium2 Bass kernel for conditioned attention.

Computes, per (b, t, h):
    scores = Q[b,t,:,h,:] @ K[b,(t+1)%T,:,h,:].T / sqrt(D) + log(OT[b,t] + eps)
    out    = softmax(scores, axis=-1) @ V[b,(t+1)%T,:,h,:]

Sharding: batch b across the 8 cores (SPMD, one batch per core).  The roll
along T stays inside a core, so no collectives are needed.

Per-core algorithm (per t, all H=8 heads):
  - load Q[t], K[t+1], V[t+1] (interleaved with a ones column -> Vext), OT[t]
  - PE-transpose Q/K head-pairs -> qT/kT laid out [d, s]; transpose OT once
  - scores^T[z,s] = kT.T @ qT   (PSUM, contraction over d=64)
  - softmax over z without a partition reduction:
      p[z,s] = (OT^T[z,s]+eps) * exp(scores^T/8)      (ACT exp + DVE mul)
      ctx[s,0:65] = p.T @ [V | 1]                     (ones col -> denominator)
      out[s,:] = ctx[:, :64] * (1/ctx[:, 64])
    This equals softmax(qk/8 + log(OT+eps)) @ V exactly (softmax shift
    invariance); exp args are bounded (~|7|) so no max-subtraction is needed.
"""

import os
from contextlib import ExitStack

import numpy as np

import concourse.bass as bass
import concourse.mybir as mybir
import concourse.tile as tile
from concourse import bacc
from concourse.bass_utils import run_bass_kernel_spmd
from concourse.masks import make_identity

T, S, H, D = 24, 128, 8, 64
HD = H * D
DX = D + 1  # V extended with a ones column
EPS = 1e-8
SCALE = 1.0 / 8.0  # 1/sqrt(64)
N_CORES = 8

F32 = mybir.dt.float32

# test.py can flip these via env to get a profile out of the run
_TRACE = bool(os.environ.get("CA_KERNEL_TRACE"))
_TMPDIR = os.environ.get("CA_KERNEL_TMPDIR")
last_run = {}


def _install_ntff_shim():
    """The agent image's antenv lacks axon_hooks; recreate it so
    run_bass_kernel_spmd(trace=True) can reach the NTFF profiler, and
    stub out the artifact upload (keep everything local)."""
    import sys
    import types

    if "antenv.axon_hooks" not in sys.modules:
        mod = types.ModuleType("antenv.axon_hooks")
        mod._hook = None
        mod.set_axon_ntff_profile_hook = lambda h: setattr(mod, "_hook", h)
        mod.get_axon_ntff_profile_hook = lambda: mod._hook
        sys.modules["antenv.axon_hooks"] = mod
        import antenv

        antenv.axon_hooks = mod
        try:
            from trn_agent_boot.trn_boot import _ntff_profile_via_ctypes

            mod._hook = _ntff_profile_via_ctypes("/opt/axon/libaxon_pjrt.so")
        except Exception as e:
            print(f"ntff hook install failed: {e}")

    import concourse.bass_utils as bu

    bu.upload_artifacts = lambda tmpdir: tmpdir


def build_bass():
    nc = bacc.Bacc(
        "TRN2",
        target_bir_lowering=False,
        debug=False,
        num_devices=N_CORES,
    )
    q = nc.dram_tensor("q", [T, S, HD], F32, kind="ExternalInput")
    k = nc.dram_tensor("k", [T, S, HD], F32, kind="ExternalInput")
    v = nc.dram_tensor("v", [T, S, HD], F32, kind="ExternalInput")
    ot = nc.dram_tensor("ot", [T, S, S], F32, kind="ExternalInput")
    out = nc.dram_tensor("out", [T, S, HD], F32, kind="ExternalOutput")

    with tile.TileContext(nc) as tc:
        with ExitStack() as ctx:
            singles = ctx.enter_context(tc.tile_pool(name="singles", bufs=1))
            loads = ctx.enter_context(tc.tile_pool(name="loads", bufs=3))
            trs = ctx.enter_context(tc.tile_pool(name="trs", bufs=2))
            sm = ctx.enter_context(tc.tile_pool(name="sm", bufs=2))
            outs = ctx.enter_context(tc.tile_pool(name="outs", bufs=3))
            ps_tr = ctx.enter_context(tc.tile_pool(name="ps_tr", bufs=1, space="PSUM"))
            ps_ot = ctx.enter_context(tc.tile_pool(name="ps_ot", bufs=1, space="PSUM"))
            ps_sc = ctx.enter_context(tc.tile_pool(name="ps_sc", bufs=1, space="PSUM"))
            ps_cx = ctx.enter_context(tc.tile_pool(name="ps_cx", bufs=3, space="PSUM"))

            identity = singles.tile([S, S], F32)
            make_identity(nc, identity)

            for t in range(T):
                t1 = (t + 1) % T

                q_tile = loads.tile([S, HD], F32, tag="q")
                nc.sync.dma_start(out=q_tile[:], in_=q[t])
                k_tile = loads.tile([S, HD], F32, tag="k")
                nc.sync.dma_start(out=k_tile[:], in_=k[t1])
                vx_tile = loads.tile([S, H, DX], F32, tag="vx")
                nc.sync.dma_start(
                    out=vx_tile[:, :, 0:D],
                    in_=v[t1].rearrange("s (h d) -> s h d", h=H),
                )
                nc.gpsimd.memset(vx_tile[:, :, D:DX], 1.0)
                ot_tile = loads.tile([S, S], F32, tag="ot")
                nc.sync.dma_start(out=ot_tile[:], in_=ot[t])

                # transpose Q and K head-pairs: [s, 2*64] -> [2*64, s]
                qT_ps = ps_tr.tile([S, HD], F32, tag="qT")
                kT_ps = ps_tr.tile([S, HD], F32, tag="kT")
                for p in range(4):
                    sl = slice(p * 128, (p + 1) * 128)
                    nc.tensor.transpose(qT_ps[:, sl], q_tile[:, sl], identity[:])
                    nc.tensor.transpose(kT_ps[:, sl], k_tile[:, sl], identity[:])
                qT = trs.tile([S, HD], F32, tag="qTs")
                kT = trs.tile([S, HD], F32, tag="kTs")
                nc.scalar.copy(qT[:], qT_ps[:])
                nc.vector.tensor_copy(kT[:], kT_ps[:])

                # OT^T (+ eps), shared across heads
                otT_ps = ps_ot.tile([S, S], F32, tag="otT")
                nc.tensor.transpose(otT_ps[:], ot_tile[:], identity[:])
                wT = sm.tile([S, S], F32, tag="wT")
                nc.vector.tensor_scalar_add(wT[:], otT_ps[:], EPS)

                # scores^T for all heads: [z, slot(h)*128 + s].
                # PE row-group tiles (even heads at array rows 0-63, odd at
                # 64-127) execute concurrently and must write DIFFERENT PSUM
                # banks: put even heads in bank 0 (cols 0-511), odd in bank 1.
                # So slot(h) = (h%2)*4 + h//2, i.e. slot order [0,2,4,6,1,3,5,7].
                sc_ps = ps_sc.tile([S, H * S], F32, tag="sc")
                for h in range(H):
                    pair, half = divmod(h, 2)
                    slot = half * 4 + pair
                    prow = slice(half * 64, half * 64 + 64)
                    pcol = slice(pair * 128, (pair + 1) * 128)
                    nc.tensor.matmul(
                        out=sc_ps[:, slot * S:(slot + 1) * S],
                        lhsT=kT[prow, pcol],
                        rhs=qT[prow, pcol],
                        start=True,
                        stop=True,
                    )

                # p[z, h, s] = (otT+eps)[z, s] * exp(scores^T / 8)
                e_sb = sm.tile([S, H, S], F32, tag="e")
                nc.scalar.activation(
                    out=e_sb[:].rearrange("z h s -> z (h s)"),
                    in_=sc_ps[:],
                    func=mybir.ActivationFunctionType.Exp,
                    scale=SCALE,
                )
                p_sb = sm.tile([S, H, S], F32, tag="p")
                wT_ap = wT[:]
                wT_b = bass.AP(
                    tensor=wT_ap.tensor,
                    offset=wT_ap.offset,
                    ap=[wT_ap.ap[0], [0, H], wT_ap.ap[1]],
                )
                nc.vector.tensor_tensor(
                    out=p_sb[:], in0=e_sb[:], in1=wT_b, op=mybir.AluOpType.mult
                )

                # ctx[s, 65] = p.T @ [V | 1]; col 64 is the softmax denominator.
                # p_sb slot j holds head (j%4)*2 + j//4 per the slot order
                # above; group jj covers heads jj, jj+2, jj+4, jj+6.
                o_tile = outs.tile([S, H, D], F32, tag="o")
                for jj in range(2):
                    cx_ps = ps_cx.tile([S, 4, DX], F32, tag="cx")
                    for hh in range(4):
                        h = hh * 2 + jj
                        nc.tensor.matmul(
                            out=cx_ps[:, hh, :],
                            lhsT=p_sb[:, jj * 4 + hh, :],
                            rhs=vx_tile[:, h, :],
                            start=True,
                            stop=True,
                        )
                    rden = sm.tile([S, 4], F32, tag="rden")
                    nc.vector.reciprocal(rden[:], cx_ps[:, :, D])
                    rden_ap = rden[:]
                    rden_b = bass.AP(
                        tensor=rden_ap.tensor,
                        offset=rden_ap.offset,
                        ap=[rden_ap.ap[0], rden_ap.ap[1], [0, D]],
                    )
                    o_grp = o_tile[:].rearrange(
                        "s (hh two) d -> s hh two d", two=2
                    )[:, :, jj, :]
                    nc.vector.tensor_tensor(
                        out=o_grp,
                        in0=cx_ps[:, :, 0:D],
                        in1=rden_b,
                        op=mybir.AluOpType.mult,
                    )

                nc.sync.dma_start(
                    out=out[t], in_=o_tile[:].rearrange("s h d -> s (h d)")
                )

    nc.compile()
    return nc


def kernel(queries, keys, values, OT_matrices):
    queries = np.ascontiguousarray(np.asarray(queries, dtype=np.float32))
    keys = np.ascontiguousarray(np.asarray(keys, dtype=np.float32))
    values = np.ascontiguousarray(np.asarray(values, dtype=np.float32))
    OT_matrices = np.ascontiguousarray(np.asarray(OT_matrices, dtype=np.float32))
    B = queries.shape[0]
    assert B == N_CORES

    if _TRACE:
        _install_ntff_shim()
    nc = build_bass()
    in_maps = [
        {
            "q": queries[b].reshape(T, S, HD),
            "k": keys[b].reshape(T, S, HD),
            "v": values[b].reshape(T, S, HD),
            "ot": OT_matrices[b],
        }
        for b in range(B)
    ]
    res = run_bass_kernel_spmd(
        nc, in_maps, list(range(N_CORES)), trace=_TRACE, tmpdir=_TMPDIR
    )
    last_run["exec_time_ns"] = res.exec_time_ns
    last_run["mean_exec_time_ns"] = res.mean_exec_time_ns
    last_run["profile_json"] = res.profile_json
    out = np.stack([res.results[b]["out"].reshape(T, S, H, D) for b in range(B)])
    return out
